# revision 2
# baseline (speedup 1.0000x reference)
"""Contrastive loss (InfoNCE-style logsumexp of cosine-similarity matrix) on
8 Trainium2 NeuronCores.

loss = -mean_i logsumexp_j( cos(z1_i, z2_j) / 0.05 ),  z1,z2: [8192, 512] f32

Strategy: shard z1 row-wise (1024 rows/core), replicate z2. Each core:
  1. loads its z1 shard + full z2, computes per-row inverse norms
     (ACT Square+accum -> Sqrt -> DVE reciprocal), scales rows
     (fold 1/0.05 into z1's scale) writing float32r (rounded fp32, 11-bit
     mantissa -> 4x faster PE datapath),
  2. transposes both to d-major layout via PE transposes (PSUM) + DVE copies,
  3. computes sim = z1h @ z2h.T as 8x16 PSUM tiles [128, 512] (f32r matmuls,
     K=512 via 4 accumulating chunks),
  4. exp in-place on PSUM with fused row-sum (ACT accum_out), logsumexp
     without max-subtraction (|sim| <= 20 so exp <= 5e8, safe in fp32),
  5. one reduce + Ln -> per-row lse [128, 8] -> DRAM.
Host gathers the 8 lse tiles and returns -mean.
"""
import sys

sys.path.insert(0, "/opt/trn_rl_repo")
import numpy as np
import concourse.bacc as bacc
import concourse.mybir as mybir
from concourse import tile, masks
from concourse.bass_utils import run_bass_kernel_spmd

F32 = mybir.dt.float32
F32R = mybir.dt.float32r
AF = mybir.ActivationFunctionType
ALU = mybir.AluOpType

N, D, C = 8192, 512, 8
NS = N // C            # 1024 z1 rows per core
IB = NS // 128         # 8 i-blocks per core
NB2 = N // 128         # 64 z2 row-blocks
JG = 16                # j-groups of 512 columns
INV_TEMP = 20.0        # 1 / 0.05


def _build():
    nc = bacc.Bacc("TRN2", target_bir_lowering=False, debug=False, num_devices=C)
    z1_d = nc.dram_tensor("z1s", [NS, D], F32, kind="ExternalInput").ap()
    z2_d = nc.dram_tensor("z2", [N, D], F32, kind="ExternalInput").ap()
    lse_d = nc.dram_tensor("lse", [128, IB], F32, kind="ExternalOutput").ap()

    with tile.TileContext(nc) as tc:
        with (
            tc.tile_pool(name="const", bufs=1) as cpool,
            tc.tile_pool(name="stage", bufs=3) as stg,
            tc.tile_pool(name="hat", bufs=3) as hat,
            tc.tile_pool(name="sqs", bufs=2) as sqs,
            tc.tile_pool(name="ptr", bufs=3, space="PSUM") as ptr,
            tc.tile_pool(name="pmm", bufs=4, space="PSUM") as pmm,
        ):
            ident_f = cpool.tile([128, 128], F32)
            masks.make_identity(nc, ident_f[:])
            ident = cpool.tile([128, 128], F32R, name="ident_r")
            nc.scalar.copy(ident[:], ident_f[:])

            z1T = cpool.tile([128, 4 * NS], F32R, name="z1T")    # [d, (k, i)]
            z2T = cpool.tile([128, 4 * N], F32R, name="z2T")     # [d, (k, j)]
            z1Tv = z1T[:].rearrange("p (k i) -> p k i", k=4)
            z2Tv = z2T[:].rearrange("p (k j) -> p k j", k=4)

            n1sq = cpool.tile([128, IB], F32, name="n1sq")
            n1s = cpool.tile([128, IB], F32, name="n1s")
            rn1 = cpool.tile([128, IB], F32, name="rn1")
            n2sq = cpool.tile([128, NB2], F32, name="n2sq")
            n2s = cpool.tile([128, NB2], F32, name="n2s")
            rn2 = cpool.tile([128, NB2], F32, name="rn2")
            esums = cpool.tile([128, IB * JG], F32, name="esums")
            stot = cpool.tile([128, IB], F32, name="stot")
            lse_s = cpool.tile([128, IB], F32, name="lse_s")

            # ---------- z1 shard: load, norms, normalize (x20), transpose
            z1r = z1_d.rearrange("(g n p) d -> g p n d", n=4, p=128)
            z1st = []
            for g in range(2):
                st = stg.tile([128, 4 * D], F32, tag="stage", name=f"st1_{g}")
                nc.sync.dma_start(out=st[:].rearrange("p (n d) -> p n d", n=4), in_=z1r[g])
                z1st.append(st)
                for n in range(4):
                    b = 4 * g + n
                    sq = sqs.tile([128, D], F32, tag="sq", name=f"sq1_{b}")
                    nc.scalar.activation(sq[:], st[:, n * D:(n + 1) * D],
                                         AF.Square, accum_out=n1sq[:, b:b + 1])
            nc.scalar.activation(n1s[:], n1sq[:], AF.Sqrt)
            nc.vector.reciprocal(rn1[:], n1s[:])
            for g in range(2):
                for n in range(4):
                    b = 4 * g + n
                    zh = hat.tile([128, D], F32R, tag="hat", name=f"zh1_{b}")
                    nc.vector.tensor_scalar(
                        zh[:], z1st[g][:, n * D:(n + 1) * D],
                        rn1[:, b:b + 1], INV_TEMP, op0=ALU.mult, op1=ALU.mult)
                    pt = ptr.tile([128, 512], F32R, tag="pt", name=f"pt1_{b}")
                    for q in range(4):
                        nc.tensor.transpose(pt[:, q * 128:(q + 1) * 128],
                                            zh[:, q * 128:(q + 1) * 128], ident[:])
                    nc.vector.tensor_copy(z1Tv[:, :, b * 128:(b + 1) * 128], pt[:])

            # ---------- z2 full: same, in groups of 4 row-blocks
            z2r = z2_d.rearrange("(g n p) d -> g p n d", n=4, p=128)
            for g in range(JG):
                st = stg.tile([128, 4 * D], F32, tag="stage", name=f"st2_{g}")
                nc.sync.dma_start(out=st[:].rearrange("p (n d) -> p n d", n=4), in_=z2r[g])
                for n in range(4):
                    b = 4 * g + n
                    sq = sqs.tile([128, D], F32, tag="sq", name=f"sq2_{b}")
                    nc.scalar.activation(sq[:], st[:, n * D:(n + 1) * D],
                                         AF.Square, accum_out=n2sq[:, b:b + 1])
                g4 = slice(4 * g, 4 * g + 4)
                nc.scalar.activation(n2s[:, g4], n2sq[:, g4], AF.Sqrt)
                nc.vector.reciprocal(rn2[:, g4], n2s[:, g4])
                for n in range(4):
                    b = 4 * g + n
                    zh = hat.tile([128, D], F32R, tag="hat", name=f"zh2_{b}")
                    nc.vector.tensor_scalar_mul(
                        zh[:], st[:, n * D:(n + 1) * D], rn2[:, b:b + 1])
                    pt = ptr.tile([128, 512], F32R, tag="pt", name=f"pt2_{b}")
                    for q in range(4):
                        nc.tensor.transpose(pt[:, q * 128:(q + 1) * 128],
                                            zh[:, q * 128:(q + 1) * 128], ident[:])
                    nc.vector.tensor_copy(z2Tv[:, :, b * 128:(b + 1) * 128], pt[:])

            # ---------- main: sim blocks + exp + row-sums
            for ib in range(IB):
                for jb in range(JG):
                    ps = pmm.tile([128, 512], F32, tag="mm", name=f"mm_{ib}_{jb}")
                    for k in range(4):
                        nc.tensor.matmul(
                            ps[:],
                            lhsT=z1Tv[:, k, ib * 128:(ib + 1) * 128],
                            rhs=z2Tv[:, k, jb * 512:(jb + 1) * 512],
                            start=(k == 0), stop=(k == 3))
                    nc.scalar.activation(ps[:], ps[:], AF.Exp,
                                         accum_out=esums[:, ib * JG + jb:ib * JG + jb + 1])

            # ---------- logsumexp tail
            nc.vector.reduce_sum(stot[:], esums[:].rearrange("p (a b) -> p a b", b=JG),
                                 axis=mybir.AxisListType.X)
            nc.scalar.activation(lse_s[:], stot[:], AF.Ln)
            nc.sync.dma_start(out=lse_d[:], in_=lse_s[:])

    nc.compile()
    return nc


_nc = None


def _get_nc():
    global _nc
    if _nc is None:
        _nc = _build()
    return _nc


def kernel(z1: np.ndarray, z2: np.ndarray, _trace: bool = False, **_):
    nc = _get_nc()
    z1 = np.ascontiguousarray(z1, dtype=np.float32)
    z2 = np.ascontiguousarray(z2, dtype=np.float32)
    in_maps = [
        {"z1s": z1[c * NS:(c + 1) * NS], "z2": z2} for c in range(C)
    ]
    res = run_bass_kernel_spmd(nc, in_maps, list(range(C)), trace=_trace)
    total = 0.0
    for c in range(C):
        total += res.results[c]["lse"].astype(np.float64).sum()
    out = np.float32(-(total / N))
    if _trace:
        return out, res
    return out
